# revision 1
# baseline (speedup 1.0000x reference)
"""Masked attention-aggregator kernel for Trainium2 (8 NeuronCores, SPMD).

Reference computation (B=16, N=2048, D=128, DQ=64), all fp32:
    q = x @ Wq.T + bq                      [B, N, DQ]
    k = x @ Wk.T + bk                      [B, N, DQ]
    s = (k @ q.T) / sqrt(DQ)               [B, N, N]   (s[b,n,m] = k[n].q[m])
    w = softmax(s + (mask[m]>0 ? 0 : -1e9), axis=m)
    out = w @ x                            [B, N, D]

Strategy: data-parallel over batch (2 batches per core).  Per batch, a
flash-style streaming attention that never materializes [N, N] anywhere.

Key transfer trick: the n axis is PERMUTED on the host so the mask-kept
columns come first.  The computation is n-equivariant (keys k[n] and
output rows out[n] just follow the permutation; the host un-permutes the
output).  The aggregated (m) axis is then simply the first mcp permuted
columns, so ONE copy of x per core serves as:
  * keys-projection source      xt[:, 0:N]
  * queries-projection source   xt[:, 0:mcp]       (same buffer, sliced)
  * numerator source            xbar-DMA-transposed [128,128] chunks of
                                xt[:, 0:mcp] (m on partitions)
Columns cnt..mcp hold *dropped* (masked-out) x values, not zeros; they
are killed by a -30000 additive penalty riding a 65th contraction row of
the scores matmul (qt row 64 = penalty row, kt row 64 = all-ones), which
drives their softmax weight to exactly 0 in both numerator and
denominator.

All tensor data travels as float16 (host-quantized): halves transfer and
SBUF/DMA traffic, doubles DVE throughput.  PSUM accumulation stays fp32;
measured end-to-end error ~5e-4 vs the 2e-2 gate.  The 1/sqrt(DQ) scale
is folded into Wq on the host.

Scores are computed transposed, ST[m, n] = q_s[m].k[n], with m on PSUM
partitions, so E^T = exp(ST) lands in SBUF (f16) already laid out as the
rhs for the numerator (xcb_chunk^T @ E^T -> out^T[d, n], PSUM fp32).
The denominator accumulates E^T chunk-wise on the DVE in f16 (2x DVE
mode; f16's 10-bit mantissa keeps the den error ~1e-3), then one ones^T
matmul per group reduces across partitions and replicates den to all
128 partitions (fp32 PSUM), so the final divide is a plain elementwise
op (reciprocal_approx_fast + multiply on the DVE).

Output is produced transposed ([D, N] per batch, f16, one store per
batch) and un-permuted / un-transposed / upcast on the host.
"""

import math
import os

import numpy as np

B, N, D, DQ = 16, 2048, 128, 64
NCORES = 8
BPC = B // NCORES  # batches per core

_cache = {}


def _build_program(mcp: int, reps: int = 1, NG: int = 1024):
    """Build the per-core Bass program for a compacted/padded m-size of mcp."""
    import concourse.bass as bass
    import concourse.tile as tile
    from concourse import bacc, mybir

    f32 = mybir.dt.float32
    f32r = mybir.dt.float32r
    f16 = mybir.dt.float16
    mc = mcp // 128  # number of m chunks
    ngroups = N // NG

    nc = bacc.Bacc("TRN2", target_bir_lowering=False, debug=False, num_devices=1)

    xt = nc.dram_tensor("xt", [BPC, D, N], f16, kind="ExternalInput").ap()
    pen = nc.dram_tensor("pen", [BPC, 1, mcp], f16, kind="ExternalInput").ap()
    onerow = nc.dram_tensor("onerow", [1, N], f16, kind="ExternalInput").ap()
    auxw = nc.dram_tensor("auxw", [D, 2 * DQ], f16, kind="ExternalInput").ap()
    bias = nc.dram_tensor("bias", [DQ, 2], f32, kind="ExternalInput").ap()
    out = nc.dram_tensor("out", [BPC, D, N], f16, kind="ExternalOutput").ap()

    with tile.TileContext(nc) as tc:
        with (
            tc.tile_pool(name="singles", bufs=1) as singles,
            tc.tile_pool(name="xtp", bufs=2) as xtp,
            tc.tile_pool(name="xcp", bufs=2) as xcp,
            tc.tile_pool(name="qtp", bufs=2) as qtp,
            tc.tile_pool(name="ktp", bufs=2) as ktp,
            tc.tile_pool(name="etp", bufs=13) as etp,
            tc.tile_pool(name="eap", bufs=3) as eap,
            tc.tile_pool(name="rdp", bufs=2) as rdp,
            tc.tile_pool(name="oevp", bufs=2) as oevp,
            tc.tile_pool(name="nrmp", bufs=2) as nrmp,
            tc.tile_pool(name="st", bufs=3, space="PSUM") as stp,
            tc.tile_pool(name="oa", bufs=1, space="PSUM") as oap,
        ):
            auxw_sb = singles.tile([D, 2 * DQ], f16)
            nc.sync.dma_start(auxw_sb[:], auxw[:])
            bias_sb = singles.tile([DQ, 2], f32)
            nc.sync.dma_start(bias_sb[:], bias[:])
            ones = singles.tile([128, 128], f16)
            nc.vector.memset(ones[:], 1.0)

            def body():
              for b in range(BPC):
                # ---- load (one copy of x per batch) ----
                xt_t = xtp.tile([D, N], f16, tag="xt")
                nc.sync.dma_start(xt_t[:, 0:N // 2], xt[b][:, 0:N // 2])
                nc.sync.dma_start(xt_t[:, N // 2:], xt[b][:, N // 2:])

                # m-chunk-major compacted x via xbar DMA transpose
                xcb_t = xcp.tile([128, mc * D], f16, tag="xc")
                nc.sync.dma_start_transpose(
                    xcb_t[:].rearrange("p (m d) -> p m d", m=mc),
                    xt_t[:, 0:mcp])

                # ---- projections (into [dq+1, m] / [dq+1, n] layout) ----
                def project_span(dst, w_col, j0, span, b_col):
                    pp = stp.tile([128, NG], f32, tag="st")
                    for j in range(0, span, 512):
                        jw = min(512, span - j)
                        nc.tensor.matmul(
                            pp[0:DQ, j:j + jw],
                            auxw_sb[:, w_col * DQ:(w_col + 1) * DQ],
                            xt_t[:, j0 + j:j0 + j + jw],
                            start=True, stop=True)
                    nc.scalar.activation(
                        dst[0:DQ, j0:j0 + span], pp[0:DQ, 0:span],
                        mybir.ActivationFunctionType.Identity,
                        bias=bias_sb[:, b_col:b_col + 1])

                qt_t = qtp.tile([DQ + 1, mcp], f16, tag="qt")
                nc.gpsimd.dma_start(qt_t[DQ:DQ + 1, :], pen[b])
                kt_t = ktp.tile([DQ + 1, N], f16, tag="kt")
                nc.gpsimd.dma_start(kt_t[DQ:DQ + 1, :], onerow[:])
                # interleave q/k projection spans so ACT's FIFO delivers the
                # first q-evac AND first k-evac (the m-loop's prerequisites)
                # before the remaining spans
                spans = []
                for j0 in range(0, mcp, NG):
                    spans.append((qt_t, 0, j0, min(NG, mcp - j0), 0))
                for i, j0 in enumerate(range(0, N, NG)):
                    spans.insert(2 * i + 1, (kt_t, 1, j0, min(NG, N - j0), 1))
                for dst, w_col, j0, span, b_col in spans:
                    project_span(dst, w_col, j0, span, b_col)

                # ---- attention over n-groups ----
                for g in range(ngroups):
                    oa = oap.tile([128, NG], f32, tag="oa")
                    eacc = eap.tile([128, NG], f16, tag="eacc")
                    for m in range(mc):
                        st = stp.tile([128, NG], f32, tag="st")
                        for h in range(NG // 512):
                            nc.tensor.matmul(
                                st[:, h * 512:(h + 1) * 512],
                                qt_t[:, m * 128:(m + 1) * 128],
                                kt_t[:, g * NG + h * 512: g * NG + (h + 1) * 512],
                                start=True, stop=True)
                        first, last = (m == 0), (m == mc - 1)
                        if first:
                            et = eacc
                        else:
                            et = etp.tile([128, NG], f16, tag="et")
                        nc.scalar.activation(et[:], st[:],
                                             mybir.ActivationFunctionType.Exp)
                        for h in range(NG // 512):
                            hs = slice(h * 512, (h + 1) * 512)
                            nc.tensor.matmul(oa[:, hs],
                                             xcb_t[:, m * D:(m + 1) * D],
                                             et[:, hs], start=first, stop=last)
                        if not first:
                            nc.vector.tensor_add(eacc[:], eacc[:], et[:])
                    # den replicated over partitions via ones.T @ eacc
                    dn = stp.tile([128, NG], f32, tag="st")
                    for h in range(NG // 512):
                        hs = slice(h * 512, (h + 1) * 512)
                        nc.tensor.matmul(dn[:, hs], ones[:], eacc[:, hs],
                                         start=True, stop=True)
                    # evacuate oa -> SBUF on ACT (2x f16) so the PSUM
                    # buffer frees before the recip/mul chain completes
                    oev = oevp.tile([128, NG], f16, tag="oev")
                    nc.scalar.activation(oev[:], oa[:],
                                         mybir.ActivationFunctionType.Identity)
                    rden = rdp.tile([128, NG], f32, tag="rden")
                    nc.vector.reciprocal_approx_fast(rden[:], dn[:])
                    nrm = nrmp.tile([128, NG], f16, tag="nrm")
                    nc.vector.tensor_mul(nrm[:], oev[:], rden[:])
                    nc.gpsimd.dma_start(out[b][:, g * NG:(g + 1) * NG], nrm[:])

            if reps > 1:
                with tc.For_i(0, reps, 1):
                    body()
            else:
                body()

    nc.compile()
    return nc


def _prep(x, mask, Wq, bq, Wk, bk):
    """Host-side prep: n-permutation (kept cols first), f16 casts, sharding."""
    x = np.asarray(x, dtype=np.float32)
    mask = np.asarray(mask)
    Wq = np.asarray(Wq, dtype=np.float32)
    bq = np.asarray(bq, dtype=np.float32)
    Wk = np.asarray(Wk, dtype=np.float32)
    bk = np.asarray(bk, dtype=np.float32)

    scale = np.float32(1.0 / math.sqrt(DQ))

    perm = np.empty((B, N), dtype=np.int64)
    counts = []
    for b in range(B):
        keep = np.nonzero(mask[b] > 0)[0]
        drop = np.nonzero(mask[b] <= 0)[0]
        perm[b, :len(keep)] = keep
        perm[b, len(keep):] = drop
        counts.append(len(keep))
    mcap = max(max(counts), 1)
    mcp = ((mcap + 127) // 128) * 128

    # x with columns permuted (kept first), transposed to [D, N], f16
    xp = np.take_along_axis(x, perm[:, :, None], axis=1)       # [B, N, D]
    xt = np.ascontiguousarray(xp.transpose(0, 2, 1)).astype(np.float16)

    pen = np.full((B, 1, mcp), -30000.0, dtype=np.float16)
    for b in range(B):
        pen[b, 0, :counts[b]] = 0.0

    auxw = np.concatenate([(Wq * scale).T, Wk.T], axis=1).astype(np.float16)
    bias = np.stack([bq * scale, bk], axis=1).astype(np.float32)  # [DQ, 2]
    one_row = np.ones((1, N), dtype=np.float16)

    in_maps = []
    for c in range(NCORES):
        s = slice(c * BPC, (c + 1) * BPC)
        in_maps.append({
            "xt": xt[s], "pen": pen[s],
            "auxw": auxw, "bias": bias, "onerow": one_row,
        })
    return in_maps, mcp, perm


def kernel(x, mask, Wq, bq, Wk, bk):
    from concourse import bass_utils

    in_maps, mcp, perm = _prep(x, mask, Wq, bq, Wk, bk)

    if mcp not in _cache:
        _cache[mcp] = _build_program(mcp)
    nc = _cache[mcp]

    res = bass_utils.run_bass_kernel_spmd(
        nc, in_maps, core_ids=list(range(NCORES)),
        trace=bool(os.environ.get("BASS_TRACE")),
    )
    kernel._last_results = res

    out_t = np.concatenate([res.results[c]["out"] for c in range(NCORES)], axis=0)
    outp = out_t.astype(np.float32).transpose(0, 2, 1)  # [B, N, D], permuted n
    out = np.empty_like(outp)
    bidx = np.arange(B)[:, None]
    out[bidx, perm] = outp  # inverse permutation of the n axis
    return np.ascontiguousarray(out)



# revision 3
# speedup vs baseline: 1.1998x; 1.1998x over previous
"""Masked attention-aggregator kernel for Trainium2 (8 NeuronCores, SPMD).

Reference computation (B=16, N=2048, D=128, DQ=64), all fp32:
    q = x @ Wq.T + bq                      [B, N, DQ]
    k = x @ Wk.T + bk                      [B, N, DQ]
    s = (k @ q.T) / sqrt(DQ)               [B, N, N]
    w = softmax(s + (mask[m]>0 ? 0 : -1e9), axis=m)
    out = w @ x                            [B, N, D]

Strategy: data-parallel over batch (2 batches per core).  Per batch, a
flash-style streaming attention that never materializes [N, N]; the host
permutes the n axis so mask-kept columns come first (the computation is
n-equivariant; the host un-permutes the output).  Dropped columns are
killed exactly by a -192 additive penalty riding a 65th contraction row
of the transposed scores matmul (qt row 64 = pen row, kt row 64 = ones),
driving their softmax weight to 0.

Engine split (found by HW bisection -- GPSIMD/Pool cannot touch PSUM on
real TRN2, and Pool tensor ops / DVE int16 converts are far slower on HW
than the cost model suggests):
  PE   : q/k projections, scores ST[m,n] (f16, PSUM f32), numerator
         out^T[d,n] += xcb_chunk^T @ E^T (PSUM accumulated over chunks),
         per-group den matmul ones^T @ (eacc + et_last) in PSUM.
  ACT  : all exp tiles (fp32 PSUM -> f16 E, bias AP), q/k evacuations
         (Copy / Identity+bias), exp table preloaded at start.
  DVE  : den accumulation (f16 2x adds), reciprocal_approx_fast, final
         divide (oa PSUM x rden -> f16).
  Pool : software-DGE DMA dispatch only (pen/ones row loads, stores).

HW-derived layout rules: dram transfers must be large and contiguous
(xt is one 512KB DMA; xcb is pre-transposed m-chunk-major on the host,
removing the on-device xbar transpose; the output is stored group-major
[BPC, ngroups, D, NG] so every store is contiguous).  Tiles are split
along span boundaries (qt main/tail, kt per group, last-chunk E halves)
because the Tile dependency tracker is per-tile, not per-range.  The
per-group den matmul folds the last chunk via PSUM accumulation so the
serial eacc add chain never gates it.  All tensor data travels as f16
(PSUM accumulation in f32); measured rel err ~5.4e-4 vs the 2e-2 gate.
"""

import math
import os

import numpy as np

B, N, D, DQ = 16, 2048, 128, 64
NCORES = 8
BPC = B // NCORES  # batches per core
NG = 1024          # n-group size

A16 = 1024.0 * 1.4426950408889634        # f16 Schraudolph multiplier
PEN_KEEP = float(np.float16(15360.0 / A16))   # = B/A in f16 (10.3984375)
PEN_DROP = PEN_KEEP - 192.0

_cache = {}
INST_LABELS = {}


def _lab(inst, label):
    try:
        INST_LABELS[inst.ins.name] = label
    except AttributeError:
        try:
            INST_LABELS[inst.name] = label
        except Exception:
            pass
    return inst


def _calibrate_act_bias():
    """ACT tiles must match the Schraudolph tiles' implied scale alpha."""
    s = np.random.default_rng(0).normal(0.0, 1.0 / 3.0, 100000).astype(np.float32)
    t = np.maximum((s + np.float32(PEN_KEEP)) * np.float32(A16), 0.0)
    bits = t.astype(np.int16)
    e = bits.view(np.float16).astype(np.float64)
    ln_alpha = np.mean(np.log(e) - s.astype(np.float64))
    return float(ln_alpha - PEN_KEEP)


ACT_EXP_BIAS = _calibrate_act_bias()

# per-m-chunk engine assignment (mc=9):  A=ACT exact exp, P=Pool, V=DVE
# Schraudolph.  Keep Schraudolph tiles a minority for accuracy.
EXP_ENG = ['A', 'A', 'A', 'A', 'A', 'A', 'A', 'A', 'A']
# den accumulation engine per m-chunk (m>=1 adds; m==0 is the eacc init)
DEN_ENG = ['-', 'V', 'V', 'V', 'V', 'V', 'V', 'V', '-']


def _build_program(mcp: int, reps: int = 1, has_bias: bool = False):
    """Per-core Bass program for a compacted/padded m-size of mcp."""
    import concourse.bass as bass
    import concourse.tile as tile
    from concourse import bacc, mybir

    f32 = mybir.dt.float32
    f16 = mybir.dt.float16
    i16 = mybir.dt.int16
    mc = mcp // 128
    ngroups = N // NG
    assert len(EXP_ENG) >= mc and len(DEN_ENG) >= mc

    nc = bacc.Bacc("TRN2", target_bir_lowering=False, debug=False, num_devices=1)

    xt = nc.dram_tensor("xt", [BPC, D, N], f16, kind="ExternalInput").ap()
    xcb = nc.dram_tensor("xcb", [BPC, 128, mc * D], f16, kind="ExternalInput").ap()
    pen = nc.dram_tensor("pen", [BPC, 1, mcp], f16, kind="ExternalInput").ap()
    onerow = nc.dram_tensor("onerow", [1, N], f16, kind="ExternalInput").ap()
    auxw = nc.dram_tensor("auxw", [D, 2 * DQ], f16, kind="ExternalInput").ap()
    bias = (nc.dram_tensor("bias", [DQ, 2], f32, kind="ExternalInput").ap()
            if has_bias else None)
    out = nc.dram_tensor("out", [BPC, N // NG, D, NG], f16,
                         kind="ExternalOutput").ap()

    with tile.TileContext(nc) as tc:
        with (
            tc.tile_pool(name="singles", bufs=1) as singles,
            tc.tile_pool(name="xtp", bufs=2) as xtp,
            tc.tile_pool(name="xcp", bufs=2) as xcp,
            tc.tile_pool(name="qtp", bufs=2) as qtp,
            tc.tile_pool(name="ktp", bufs=2) as ktp,
            tc.tile_pool(name="etp", bufs=6) as etp,
            tc.tile_pool(name="eap", bufs=2) as eap,
            tc.tile_pool(name="rdp", bufs=4) as rdp,
            tc.tile_pool(name="nrmp", bufs=4) as nrmp,
            tc.tile_pool(name="st", bufs=3, space="PSUM") as stp,
            tc.tile_pool(name="oa", bufs=1, space="PSUM") as oap,
        ):
            auxw_sb = singles.tile([D, 2 * DQ], f16)
            nc.sync.dma_start(auxw_sb[:], auxw[:])
            ones = singles.tile([128, 128], f16)
            nc.vector.memset(ones[:], 1.0)
            if has_bias:
                bias_sb = singles.tile([DQ, 2], f32)
                nc.sync.dma_start(bias_sb[:], bias[:])
            ebias = singles.tile([128, 1], f32)
            nc.vector.memset(ebias[:], ACT_EXP_BIAS)
            # warm the ACT exp table while the first loads are in flight
            dummy = singles.tile([128, 1], f16)
            nc.scalar.activation(dummy[:], ebias[:],
                                 mybir.ActivationFunctionType.Exp)


            def body():
              xts, xcbs, qts, kts = [], [], [], []
              for b in range(BPC):
                # ---- loads (all batches up front; SP queue is FIFO) ----
                xt_t = xtp.tile([D, N], f16, tag="xt")
                nc.sync.dma_start(xt_t[:], xt[b][:])
                qt_m = qtp.tile([DQ + 1, min(NG, mcp)], f16, tag="qt", name="qt_m")
                nc.gpsimd.dma_start(qt_m[DQ:DQ + 1, :], pen[b][:, 0:min(NG, mcp)])
                xcb_t = xcp.tile([128, mc * D], f16, tag="xc")
                nc.sync.dma_start(xcb_t[:], xcb[b][:])
                kt_gs = []
                for gg in range(ngroups):
                    kt_g = ktp.tile([DQ + 1, NG], f16, tag="kt", name="kt_g")
                    nc.gpsimd.dma_start(kt_g[DQ:DQ + 1, :],
                                      onerow[:, gg * NG:(gg + 1) * NG])
                    kt_gs.append(kt_g)
                qt_x = None
                if mcp > NG:
                    qt_x = qtp.tile([DQ + 1, mcp - NG], f16, tag="qtx", name="qt_x")
                    nc.gpsimd.dma_start(qt_x[DQ:DQ + 1, :], pen[b][:, NG:mcp])
                xts.append(xt_t); xcbs.append(xcb_t)
                qts.append((qt_m, qt_x)); kts.append(kt_gs)
              for b in range(BPC):
                xt_t, xcb_t = xts[b], xcbs[b]
                (qt_m, qt_x), kt_gs = qts[b], kts[b]

                # ---- projections into qt [65, mcp] / kt [65, N] ----
                def project_span(dst, w_col, j0, span, b_col, eng):
                    pp = stp.tile([128, NG], f32, tag="st")
                    for j in range(0, span, 512):
                        jw = min(512, span - j)
                        _lab(nc.tensor.matmul(
                            pp[0:DQ, j:j + jw],
                            auxw_sb[:, w_col * DQ:(w_col + 1) * DQ],
                            xt_t[:, j0 + j:j0 + j + jw],
                            start=True, stop=True), f"b{b}.proj{w_col}.j{j0+j}")
                    if has_bias:
                        _lab(nc.scalar.activation(
                            dst[0:DQ, 0:span], pp[0:DQ, 0:span],
                            mybir.ActivationFunctionType.Identity,
                            bias=bias_sb[:, b_col:b_col + 1]),
                             f"b{b}.evac{w_col}.j{j0}")
                    elif eng == 'A':
                        _lab(nc.scalar.activation(
                            dst[0:DQ, 0:span], pp[0:DQ, 0:span],
                            mybir.ActivationFunctionType.Copy),
                             f"b{b}.evac{w_col}.j{j0}")
                    else:
                        _lab(nc.vector.tensor_copy(
                            dst[0:DQ, 0:span], pp[0:DQ, 0:span]),
                             f"b{b}.evac{w_col}.j{j0}")

                # q.j0 and k.j0 evac first (on separate engines) so the first
                # 8 score chunks can start; the tail spans follow.
                spans = [(qt_m, 0, 0, min(NG, mcp), 0, 'A'),
                         (kt_gs[0], 1, 0, NG, 1, 'A'),
                         (kt_gs[1], 1, NG, N - NG, 1, 'A')]
                if mcp > NG:
                    spans.append((qt_x, 0, NG, mcp - NG, 0, 'A'))
                for dst, w_col, j0, span, b_col, eng in spans:
                    project_span(dst, w_col, j0, span, b_col, eng)

                # ---- attention over n-groups ----
                # Flat software pipeline across groups: scores/exp for item
                # i+1 are emitted before the numerator matmul of item i, so
                # the next group's scores can hide the last chunk's exp
                # latency at every group boundary.
                state = {}
                def new_group(g):
                    oa = oap.tile([128, NG], f32, tag="oa")
                    eacc = eap.tile([128, NG], f16, tag="eacc")
                    ets = [None] * mc
                    state[g] = (oa, eacc, ets)

                def scores_exp(g, m):
                    oa, eacc, ets = state[g]
                    st = stp.tile([128, NG], f32, tag="st")
                    qsrc = qt_m if m * 128 < NG else qt_x
                    qoff = m * 128 if m * 128 < NG else m * 128 - NG
                    for h in range(NG // 512):
                        _lab(nc.tensor.matmul(
                            st[:, h * 512:(h + 1) * 512],
                            qsrc[:, qoff:qoff + 128],
                            kt_gs[g][:, h * 512:(h + 1) * 512],
                            start=True, stop=True), f"b{b}.g{g}.S{m}.h{h}")
                    if m == mc - 1:
                        eth0 = etp.tile([128, NG // 2], f16, tag="et", name="eth0")
                        eth1 = etp.tile([128, NG // 2], f16, tag="et", name="eth1")
                        et = [eth0, eth1]
                    else:
                        et = eacc if m == 0 else etp.tile([128, NG], f16, tag="et")
                    ets[m] = et
                    parts = et if isinstance(et, list) else [et]
                    hsplit = len(parts)
                    for eh, ep in enumerate(parts):
                        es = slice(eh * (NG // hsplit), (eh + 1) * (NG // hsplit))
                        if EXP_ENG[m] == 'A':
                            _lab(nc.scalar.activation(
                                ep[:], st[:, es],
                                mybir.ActivationFunctionType.Exp,
                                bias=ebias[:]), f"b{b}.g{g}.E{m}.{eh}")
                        else:
                            _lab(nc.vector.tensor_scalar(
                                ep[:].bitcast(i16), st[:, es], A16, 0.0,
                                mybir.AluOpType.mult, mybir.AluOpType.max),
                                 f"b{b}.g{g}.E{m}.{eh}")

                def numer_den(g, m):
                    oa, eacc, ets = state[g]
                    first, last = (m == 0), (m == mc - 1)
                    et = ets[m]
                    for h in range(NG // 512):
                        hs = slice(h * 512, (h + 1) * 512)
                        rhs = et[h][:] if isinstance(et, list) else et[:, hs]
                        _lab(nc.tensor.matmul(oa[:, hs],
                                         xcb_t[:, m * D:(m + 1) * D],
                                         rhs, start=first, stop=last),
                             f"b{b}.g{g}.N{m}.h{h}")
                    if not first and not last:
                        deng = nc.vector if DEN_ENG[m] == 'V' else nc.gpsimd
                        _lab(deng.tensor_add(eacc[:], eacc[:], et[:]),
                             f"b{b}.g{g}.D{m}")

                def finish_group(g):
                    oa, eacc, ets = state[g]
                    # den = ones.T @ (eacc + et_last), accumulated in PSUM so
                    # the last chunk's E never enters the serial eacc chain
                    dn = stp.tile([128, NG], f32, tag="st")
                    for h in range(NG // 512):
                        hs = slice(h * 512, (h + 1) * 512)
                        _lab(nc.tensor.matmul(dn[:, hs], ones[:], eacc[:, hs],
                                         start=True, stop=False), f"b{b}.g{g}.dn{h}")
                        _lab(nc.tensor.matmul(dn[:, hs], ones[:], ets[mc - 1][h][:],
                                         start=False, stop=True), f"b{b}.g{g}.dn{h}b")
                    nmf = nrmp.tile([128, NG], f16, tag="nrm", name="nmf")
                    rdf = rdp.tile([128, NG], f32, tag="rden", name="rdf")
                    _lab(nc.vector.reciprocal_approx_fast(rdf[:], dn[:]),
                         f"b{b}.g{g}.rcp")
                    _lab(nc.vector.tensor_mul(nmf[:], oa[:], rdf[:]),
                         f"b{b}.g{g}.nrm")
                    _lab(nc.gpsimd.dma_start(out[b][g], nmf[:]),
                         f"b{b}.g{g}.store")

                items = [(g, m) for g in range(ngroups) for m in range(mc)]
                new_group(0)
                scores_exp(*items[0])
                for i in range(1, len(items)):
                    g, m = items[i]
                    if m == 0:
                        new_group(g)
                    scores_exp(g, m)
                    pg, pm = items[i - 1]
                    numer_den(pg, pm)
                    if pm == mc - 1:
                        finish_group(pg)
                numer_den(*items[-1])
                finish_group(items[-1][0])

            if reps > 1:
                with tc.For_i(0, reps, 1):
                    body()
            else:
                body()

    nc.compile()
    return nc


def _prep(x, mask, Wq, bq, Wk, bk):
    """Host-side prep: n-permutation (kept cols first), f16 casts, sharding."""
    x = np.asarray(x, dtype=np.float32)
    mask = np.asarray(mask)
    Wq = np.asarray(Wq, dtype=np.float32)
    bq = np.asarray(bq, dtype=np.float32)
    Wk = np.asarray(Wk, dtype=np.float32)
    bk = np.asarray(bk, dtype=np.float32)

    scale = np.float32(1.0 / math.sqrt(DQ))

    perm = np.empty((B, N), dtype=np.int64)
    counts = []
    for b in range(B):
        keep = np.nonzero(mask[b] > 0)[0]
        drop = np.nonzero(mask[b] <= 0)[0]
        perm[b, :len(keep)] = keep
        perm[b, len(keep):] = drop
        counts.append(len(keep))
    mcap = max(max(counts), 1)
    mcp = ((mcap + 127) // 128) * 128
    mc = mcp // 128

    # x with columns permuted (kept first), transposed to [D, N], f16
    xp = np.take_along_axis(x, perm[:, :, None], axis=1)       # [B, N, D]
    xt = np.ascontiguousarray(xp.transpose(0, 2, 1)).astype(np.float16)

    # m-chunk-major compacted x (host transpose): [128, mc*D]
    xcb = np.ascontiguousarray(
        xp[:, :mcp].reshape(B, mc, 128, D).transpose(0, 2, 1, 3)
    ).reshape(B, 128, mc * D).astype(np.float16)

    pen = np.full((B, 1, mcp), PEN_DROP, dtype=np.float16)
    for b in range(B):
        pen[b, 0, :counts[b]] = PEN_KEEP

    has_bias = bool(bq.any() or bk.any())
    auxw = np.concatenate([(Wq * scale).T, Wk.T], axis=1).astype(np.float16)
    bias = np.stack([bq * scale, bk], axis=1).astype(np.float32)  # [DQ, 2]
    one_row = np.ones((1, N), dtype=np.float16)

    in_maps = []
    for c in range(NCORES):
        s = slice(c * BPC, (c + 1) * BPC)
        m = {
            "xt": xt[s], "xcb": xcb[s], "pen": pen[s],
            "auxw": auxw, "onerow": one_row,
        }
        if has_bias:
            m["bias"] = bias
        in_maps.append(m)
    return in_maps, mcp, perm, has_bias


def kernel(x, mask, Wq, bq, Wk, bk):
    from concourse import bass_utils

    in_maps, mcp, perm, has_bias = _prep(x, mask, Wq, bq, Wk, bk)

    key = (mcp, has_bias)
    if key not in _cache:
        _cache[key] = _build_program(mcp, has_bias=has_bias)
    nc = _cache[key]

    res = bass_utils.run_bass_kernel_spmd(
        nc, in_maps, core_ids=list(range(NCORES)),
        trace=bool(os.environ.get("BASS_TRACE")),
    )
    kernel._last_results = res

    out_t = np.concatenate([res.results[c]["out"] for c in range(NCORES)], axis=0)
    out_t = out_t.transpose(0, 2, 1, 3).reshape(B, D, N)  # [B, D, N]
    outp = out_t.astype(np.float32).transpose(0, 2, 1)  # [B, N, D], permuted n
    out = np.empty_like(outp)
    bidx = np.arange(B)[:, None]
    out[bidx, perm] = outp
    return np.ascontiguousarray(out)


# revision 4
# speedup vs baseline: 1.2108x; 1.0091x over previous
"""Masked attention-aggregator kernel for Trainium2 (8 NeuronCores, SPMD).

Reference computation (B=16, N=2048, D=128, DQ=64), all fp32:
    q = x @ Wq.T + bq                      [B, N, DQ]
    k = x @ Wk.T + bk                      [B, N, DQ]
    s = (k @ q.T) / sqrt(DQ)               [B, N, N]
    w = softmax(s + (mask[m]>0 ? 0 : -1e9), axis=m)
    out = w @ x                            [B, N, D]

Strategy: data-parallel over batch (2 batches per core).  Per batch, a
flash-style streaming attention that never materializes [N, N]; the host
permutes the n axis so mask-kept columns come first (the computation is
n-equivariant; the host un-permutes the output).  Dropped columns are
killed exactly by a -192 additive penalty riding a 65th contraction row
of the transposed scores matmul (qt row 64 = pen row, kt row 64 = ones),
driving their softmax weight to 0.

Engine split (found by HW bisection -- GPSIMD/Pool cannot touch PSUM on
real TRN2, and Pool tensor ops / DVE int16 converts are far slower on HW
than the cost model suggests):
  PE   : q/k projections, scores ST[m,n] (f16, PSUM f32), numerator
         out^T[d,n] += xcb_chunk^T @ E^T (PSUM accumulated over chunks),
         per-group den matmul ones^T @ (eacc + et_last) in PSUM.
  ACT  : all exp tiles (fp32 PSUM -> f16 E, bias AP), q/k evacuations
         (Copy / Identity+bias), exp table preloaded at start.
  DVE  : den accumulation (f16 2x adds), reciprocal_approx_fast, final
         divide (oa PSUM x rden -> f16).
  Pool : software-DGE DMA dispatch only (pen/ones row loads, stores).

HW-derived layout rules: dram transfers must be large and contiguous
(xt is one 512KB DMA; xcb is pre-transposed m-chunk-major on the host,
removing the on-device xbar transpose; the output is stored group-major
[BPC, ngroups, D, NG] so every store is contiguous).  Tiles are split
along span boundaries (qt main/tail, kt per group, last-chunk E halves)
because the Tile dependency tracker is per-tile, not per-range.  The
per-group den matmul folds the last chunk via PSUM accumulation so the
serial eacc add chain never gates it.  All tensor data travels as f16
(PSUM accumulation in f32); measured rel err ~5.4e-4 vs the 2e-2 gate.
"""

import math
import os

import numpy as np

B, N, D, DQ = 16, 2048, 128, 64
NCORES = 8
BPC = B // NCORES  # batches per core
NG = 1024          # n-group size

A16 = 1024.0 * 1.4426950408889634        # f16 Schraudolph multiplier
PEN_KEEP = float(np.float16(15360.0 / A16))   # = B/A in f16 (10.3984375)
PEN_DROP = PEN_KEEP - 192.0

_cache = {}
INST_LABELS = {}


def _lab(inst, label):
    try:
        INST_LABELS[inst.ins.name] = label
    except AttributeError:
        try:
            INST_LABELS[inst.name] = label
        except Exception:
            pass
    return inst


def _calibrate_act_bias():
    """ACT tiles must match the Schraudolph tiles' implied scale alpha."""
    s = np.random.default_rng(0).normal(0.0, 1.0 / 3.0, 100000).astype(np.float32)
    t = np.maximum((s + np.float32(PEN_KEEP)) * np.float32(A16), 0.0)
    bits = t.astype(np.int16)
    e = bits.view(np.float16).astype(np.float64)
    ln_alpha = np.mean(np.log(e) - s.astype(np.float64))
    return float(ln_alpha - PEN_KEEP)


ACT_EXP_BIAS = _calibrate_act_bias()

# per-m-chunk engine assignment (mc=9):  A=ACT exact exp, P=Pool, V=DVE
# Schraudolph.  Keep Schraudolph tiles a minority for accuracy.
EXP_ENG = ['A'] * 16
# den accumulation engine per m-chunk (m>=1 adds; m==0 is the eacc init)
DEN_ENG = ['-'] + ['V'] * 15


def _build_program(mcp: int, reps: int = 1, has_bias: bool = False):
    """Per-core Bass program for a compacted/padded m-size of mcp."""
    import concourse.bass as bass
    import concourse.tile as tile
    from concourse import bacc, mybir

    f32 = mybir.dt.float32
    f16 = mybir.dt.float16
    i16 = mybir.dt.int16
    mc = mcp // 128
    ngroups = N // NG
    assert len(EXP_ENG) >= mc and len(DEN_ENG) >= mc

    nc = bacc.Bacc("TRN2", target_bir_lowering=False, debug=False, num_devices=1)

    xt = nc.dram_tensor("xt", [BPC, D, N], f16, kind="ExternalInput").ap()
    xcb = nc.dram_tensor("xcb", [BPC, 128, mc * D], f16, kind="ExternalInput").ap()
    pen = nc.dram_tensor("pen", [BPC, 1, mcp], f16, kind="ExternalInput").ap()
    onerow = nc.dram_tensor("onerow", [1, N], f16, kind="ExternalInput").ap()
    auxw = nc.dram_tensor("auxw", [D, 2 * DQ], f16, kind="ExternalInput").ap()
    bias = (nc.dram_tensor("bias", [DQ, 2], f32, kind="ExternalInput").ap()
            if has_bias else None)
    out = nc.dram_tensor("out", [BPC, N // NG, D, NG], f16,
                         kind="ExternalOutput").ap()

    with tile.TileContext(nc) as tc:
        with (
            tc.tile_pool(name="singles", bufs=1) as singles,
            tc.tile_pool(name="xtp", bufs=2) as xtp,
            tc.tile_pool(name="xcp", bufs=2) as xcp,
            tc.tile_pool(name="qtp", bufs=2) as qtp,
            tc.tile_pool(name="ktp", bufs=2) as ktp,
            tc.tile_pool(name="etp", bufs=6) as etp,
            tc.tile_pool(name="eap", bufs=2) as eap,
            tc.tile_pool(name="rdp", bufs=4) as rdp,
            tc.tile_pool(name="nrmp", bufs=4) as nrmp,
            tc.tile_pool(name="st", bufs=3, space="PSUM") as stp,
            tc.tile_pool(name="oa", bufs=1, space="PSUM") as oap,
        ):
            auxw_sb = singles.tile([D, 2 * DQ], f16)
            nc.sync.dma_start(auxw_sb[:], auxw[:])
            ones = singles.tile([128, 128], f16)
            nc.vector.memset(ones[:], 1.0)
            if has_bias:
                bias_sb = singles.tile([DQ, 2], f32)
                nc.sync.dma_start(bias_sb[:], bias[:])
            ebias = singles.tile([128, 1], f32)
            nc.vector.memset(ebias[:], ACT_EXP_BIAS)
            # warm the ACT exp table while the first loads are in flight
            dummy = singles.tile([128, 1], f16)
            nc.scalar.activation(dummy[:], ebias[:],
                                 mybir.ActivationFunctionType.Exp)


            def body():
              xts, xcbs, qts, kts = [], [], [], []
              for b in range(BPC):
                # ---- loads (all batches up front; SP queue is FIFO) ----
                xt_t = xtp.tile([D, N], f16, tag="xt")
                nc.sync.dma_start(xt_t[:], xt[b][:])
                qt_m = qtp.tile([DQ + 1, min(NG, mcp)], f16, tag="qt", name="qt_m")
                nc.gpsimd.dma_start(qt_m[DQ:DQ + 1, :], pen[b][:, 0:min(NG, mcp)])
                xcb_t = xcp.tile([128, mc * D], f16, tag="xc")
                nc.sync.dma_start(xcb_t[:], xcb[b][:])
                kt_gs = []
                for gg in range(ngroups):
                    kt_g = ktp.tile([DQ + 1, NG], f16, tag="kt", name="kt_g")
                    nc.gpsimd.dma_start(kt_g[DQ:DQ + 1, :],
                                      onerow[:, gg * NG:(gg + 1) * NG])
                    kt_gs.append(kt_g)
                qt_x = None
                if mcp > NG:
                    qt_x = qtp.tile([DQ + 1, mcp - NG], f16, tag="qtx", name="qt_x")
                    nc.gpsimd.dma_start(qt_x[DQ:DQ + 1, :], pen[b][:, NG:mcp])
                xts.append(xt_t); xcbs.append(xcb_t)
                qts.append((qt_m, qt_x)); kts.append(kt_gs)
              for b in range(BPC):
                xt_t, xcb_t = xts[b], xcbs[b]
                (qt_m, qt_x), kt_gs = qts[b], kts[b]

                # ---- projections into qt [65, mcp] / kt [65, N] ----
                def project_span(dst, w_col, j0, span, b_col, eng):
                    pp = stp.tile([128, NG], f32, tag="st")
                    for j in range(0, span, 512):
                        jw = min(512, span - j)
                        _lab(nc.tensor.matmul(
                            pp[0:DQ, j:j + jw],
                            auxw_sb[:, w_col * DQ:(w_col + 1) * DQ],
                            xt_t[:, j0 + j:j0 + j + jw],
                            start=True, stop=True), f"b{b}.proj{w_col}.j{j0+j}")
                    if has_bias:
                        _lab(nc.scalar.activation(
                            dst[0:DQ, 0:span], pp[0:DQ, 0:span],
                            mybir.ActivationFunctionType.Identity,
                            bias=bias_sb[:, b_col:b_col + 1]),
                             f"b{b}.evac{w_col}.j{j0}")
                    elif eng == 'A':
                        _lab(nc.scalar.activation(
                            dst[0:DQ, 0:span], pp[0:DQ, 0:span],
                            mybir.ActivationFunctionType.Copy),
                             f"b{b}.evac{w_col}.j{j0}")
                    else:
                        _lab(nc.vector.tensor_copy(
                            dst[0:DQ, 0:span], pp[0:DQ, 0:span]),
                             f"b{b}.evac{w_col}.j{j0}")

                # q.j0 and k.j0 evac first (on separate engines) so the first
                # 8 score chunks can start; the tail spans follow.
                spans = [(qt_m, 0, 0, min(NG, mcp), 0, 'A'),
                         (kt_gs[0], 1, 0, NG, 1, 'A'),
                         (kt_gs[1], 1, NG, N - NG, 1, 'A')]
                if mcp > NG:
                    spans.append((qt_x, 0, NG, mcp - NG, 0, 'A'))
                for dst, w_col, j0, span, b_col, eng in spans:
                    project_span(dst, w_col, j0, span, b_col, eng)

                # ---- attention over n-groups ----
                # Flat software pipeline across groups: scores/exp for item
                # i+1 are emitted before the numerator matmul of item i, so
                # the next group's scores can hide the last chunk's exp
                # latency at every group boundary.
                state = {}
                def new_group(g):
                    oa = oap.tile([128, NG], f32, tag="oa")
                    eacc = eap.tile([128, NG], f16, tag="eacc")
                    ets = [None] * mc
                    state[g] = (oa, eacc, ets)

                def scores_exp(g, m):
                    oa, eacc, ets = state[g]
                    st = stp.tile([128, NG], f32, tag="st")
                    qsrc = qt_m if m * 128 < NG else qt_x
                    qoff = m * 128 if m * 128 < NG else m * 128 - NG
                    for h in range(NG // 512):
                        _lab(nc.tensor.matmul(
                            st[:, h * 512:(h + 1) * 512],
                            qsrc[:, qoff:qoff + 128],
                            kt_gs[g][:, h * 512:(h + 1) * 512],
                            start=True, stop=True), f"b{b}.g{g}.S{m}.h{h}")
                    if m == mc - 1:
                        eth0 = etp.tile([128, NG // 2], f16, tag="et", name="eth0")
                        eth1 = etp.tile([128, NG // 2], f16, tag="et", name="eth1")
                        et = [eth0, eth1]
                    else:
                        et = eacc if m == 0 else etp.tile([128, NG], f16, tag="et")
                    ets[m] = et
                    parts = et if isinstance(et, list) else [et]
                    hsplit = len(parts)
                    for eh, ep in enumerate(parts):
                        es = slice(eh * (NG // hsplit), (eh + 1) * (NG // hsplit))
                        if EXP_ENG[m] == 'A':
                            _lab(nc.scalar.activation(
                                ep[:], st[:, es],
                                mybir.ActivationFunctionType.Exp,
                                bias=ebias[:]), f"b{b}.g{g}.E{m}.{eh}")
                        else:
                            _lab(nc.vector.tensor_scalar(
                                ep[:].bitcast(i16), st[:, es], A16, 0.0,
                                mybir.AluOpType.mult, mybir.AluOpType.max),
                                 f"b{b}.g{g}.E{m}.{eh}")

                def numer_den(g, m):
                    oa, eacc, ets = state[g]
                    first, last = (m == 0), (m == mc - 1)
                    et = ets[m]
                    for h in range(NG // 512):
                        hs = slice(h * 512, (h + 1) * 512)
                        rhs = et[h][:] if isinstance(et, list) else et[:, hs]
                        _lab(nc.tensor.matmul(oa[:, hs],
                                         xcb_t[:, m * D:(m + 1) * D],
                                         rhs, start=first, stop=last),
                             f"b{b}.g{g}.N{m}.h{h}")
                    if not first and not last:
                        deng = nc.vector if DEN_ENG[m] == 'V' else nc.gpsimd
                        _lab(deng.tensor_add(eacc[:], eacc[:], et[:]),
                             f"b{b}.g{g}.D{m}")

                def finish_group(g):
                    oa, eacc, ets = state[g]
                    # den = ones.T @ (eacc + et_last), accumulated in PSUM so
                    # the last chunk's E never enters the serial eacc chain
                    dn = stp.tile([128, NG], f32, tag="st")
                    for h in range(NG // 512):
                        hs = slice(h * 512, (h + 1) * 512)
                        _lab(nc.tensor.matmul(dn[:, hs], ones[:], eacc[:, hs],
                                         start=True, stop=False), f"b{b}.g{g}.dn{h}")
                        _lab(nc.tensor.matmul(dn[:, hs], ones[:], ets[mc - 1][h][:],
                                         start=False, stop=True), f"b{b}.g{g}.dn{h}b")
                    nmf = nrmp.tile([128, NG], f16, tag="nrm", name="nmf")
                    rdf = rdp.tile([128, NG], f32, tag="rden", name="rdf")
                    _lab(nc.vector.reciprocal_approx_fast(rdf[:], dn[:]),
                         f"b{b}.g{g}.rcp")
                    _lab(nc.vector.tensor_mul(nmf[:], oa[:], rdf[:]),
                         f"b{b}.g{g}.nrm")
                    _lab(nc.gpsimd.dma_start(out[b][g], nmf[:]),
                         f"b{b}.g{g}.store")

                items = [(g, m) for g in range(ngroups) for m in range(mc)]
                new_group(0)
                scores_exp(*items[0])
                for i in range(1, len(items)):
                    g, m = items[i]
                    if m == 0:
                        new_group(g)
                    scores_exp(g, m)
                    pg, pm = items[i - 1]
                    numer_den(pg, pm)
                    if pm == mc - 1:
                        finish_group(pg)
                numer_den(*items[-1])
                finish_group(items[-1][0])

            if reps > 1:
                with tc.For_i(0, reps, 1):
                    body()
            else:
                body()

    nc.compile()
    return nc


def _prep(x, mask, Wq, bq, Wk, bk):
    """Host-side prep: n-permutation (kept cols first), f16 casts, sharding."""
    x = np.asarray(x, dtype=np.float32)
    mask = np.asarray(mask)
    Wq = np.asarray(Wq, dtype=np.float32)
    bq = np.asarray(bq, dtype=np.float32)
    Wk = np.asarray(Wk, dtype=np.float32)
    bk = np.asarray(bk, dtype=np.float32)

    scale = np.float32(1.0 / math.sqrt(DQ))

    perm = np.empty((B, N), dtype=np.int64)
    counts = []
    for b in range(B):
        keep = np.nonzero(mask[b] > 0)[0]
        drop = np.nonzero(mask[b] <= 0)[0]
        perm[b, :len(keep)] = keep
        perm[b, len(keep):] = drop
        counts.append(len(keep))
    mcap = max(max(counts), 1)
    mcp = ((mcap + 127) // 128) * 128
    mc = mcp // 128

    # x with columns permuted (kept first), transposed to [D, N], f16
    xp = np.take_along_axis(x, perm[:, :, None], axis=1)       # [B, N, D]
    xt = np.ascontiguousarray(xp.transpose(0, 2, 1)).astype(np.float16)

    # m-chunk-major compacted x (host transpose): [128, mc*D]
    xcb = np.ascontiguousarray(
        xp[:, :mcp].reshape(B, mc, 128, D).transpose(0, 2, 1, 3)
    ).reshape(B, 128, mc * D).astype(np.float16)

    pen = np.full((B, 1, mcp), PEN_DROP, dtype=np.float16)
    for b in range(B):
        pen[b, 0, :counts[b]] = PEN_KEEP

    has_bias = bool(bq.any() or bk.any())
    auxw = np.concatenate([(Wq * scale).T, Wk.T], axis=1).astype(np.float16)
    bias = np.stack([bq * scale, bk], axis=1).astype(np.float32)  # [DQ, 2]
    one_row = np.ones((1, N), dtype=np.float16)

    in_maps = []
    for c in range(NCORES):
        s = slice(c * BPC, (c + 1) * BPC)
        m = {
            "xt": xt[s], "xcb": xcb[s], "pen": pen[s],
            "auxw": auxw, "onerow": one_row,
        }
        if has_bias:
            m["bias"] = bias
        in_maps.append(m)
    return in_maps, mcp, perm, has_bias


def kernel(x, mask, Wq, bq, Wk, bk):
    from concourse import bass_utils

    in_maps, mcp, perm, has_bias = _prep(x, mask, Wq, bq, Wk, bk)

    key = (mcp, has_bias)
    if key not in _cache:
        _cache[key] = _build_program(mcp, has_bias=has_bias)
    nc = _cache[key]

    res = bass_utils.run_bass_kernel_spmd(
        nc, in_maps, core_ids=list(range(NCORES)),
        trace=bool(os.environ.get("BASS_TRACE")),
    )
    kernel._last_results = res

    out_t = np.concatenate([res.results[c]["out"] for c in range(NCORES)], axis=0)
    out_t = out_t.transpose(0, 2, 1, 3).reshape(B, D, N)  # [B, D, N]
    outp = out_t.astype(np.float32).transpose(0, 2, 1)  # [B, N, D], permuted n
    out = np.empty_like(outp)
    bidx = np.arange(B)[:, None]
    out[bidx, perm] = outp
    return np.ascontiguousarray(out)
